# revision 22
# baseline (speedup 1.0000x reference)
"""Trainium2 Bass kernel for nn_Attention_3607772529228 (sparse_attention).

Reference computation (B=64, S=512, T=32, 2H=1024, ATT=512):
    ht_mean = mean(ht, axis=1)                               [B, 2H]
    z       = [h ; ht_mean] @ w1_w.T + w1_b                  [B, S, ATT]
    a       = tanh(z)
    beta    = a @ u_w[0];  beta = where(mask, beta, -1e20)   [B, S]
    alpha   = softmax(beta, axis=1)
    out     = einsum('bs,bsd->bd', alpha, h)                 [B, 2H]

Algebraic simplifications (exact):
  * where(valid, ..., 0) on h_cat / a does not affect the output (invalid
    positions only enter through beta, overwritten with -1e20).
  * The ht_mean half of the big matmul folds into a per-batch bias:
    z = h @ w1.T + (w2 @ ht_mean + w1_b).

Distribution: data-parallel over batch B across 8 cores (8 batches/core).

v3 design (the bf16 baseline is HBM-DMA-bound at ~19MB/core/rep):
  * z path in fp8-e4m3: h_t, w1, w2 shipped fp8 (weights pre-scaled x64 to
    dodge the e4m3 subnormal range; folded back via the tanh activation
    scale). z matmuls use DoubleRow (2 fp8 weights/cell, K=256/pass).
    Final rel err ~0.9e-2 (validated vs 2e-2 budget).
  * h_nat (weighted-sum copy) stays bf16 -- output precision needs it.
  * All big DMAs are host-prepped fully contiguous per partition.
  * Constants (w1/w2/htt/u/mask) + the bias matmuls are hoisted out of
    the rep loop and issued before the h_nat stream.
  * beta lands in a 32x-replicated [128, S] layout (4 batches x 32
    replicas); softmax runs in that layout (no gather DMAs). The -1e20
    mask add is one extra identity-lhsT matmul accumulated into beta's
    PSUM. Exp writes bf16 directly (f32 row-sum via accum_out); the
    1/sum normalization is deferred to the output copy (per-partition
    scale), keeping the beta->alpha->transpose chain short.
  * Per-group tail pipelining: group 0's beta/softmax/wsum PE work is
    emitted between group 1's z matmuls; only group 1's tail is exposed.
  * PE warmup matmuls only on rep 0 (cold path).
"""

import os
from contextlib import ExitStack

import numpy as np
import ml_dtypes

import concourse.bass as bass
import concourse.tile as tile
from concourse import bacc, mybir
from concourse import bass_utils
from concourse.masks import make_identity

BF16 = mybir.dt.bfloat16
F8 = mybir.dt.float8e4
F32 = mybir.dt.float32

B, S, T, H2, ATT = 64, 512, 32, 1024, 512
NCORES = 8
BL = B // NCORES  # 8 batches per core
P = 128
KC = H2 // P  # 8 k-chunks over hidden
KD = KC // 2  # 4 DoubleRow k-pairs
TT = ATT // P  # 4 attention tiles
SC = S // P  # 4 sequence chunks
NH = H2 // 512  # 2 output halves
NG = BL // 4  # batch groups of 4 (PE column-group packing)
WARMUP_MMS = 12
FP8_Z = True  # fp8 z path (h_t/w1/w2) with DoubleRow matmuls
WS = 64.0 if FP8_Z else 1.0  # fp8 weight pre-scale (dodges e4m3 subnormals)

ZDT = F8 if FP8_Z else BF16
NP_F8 = ml_dtypes.float8_e4m3


def _body(tc, reps=1):
    nc = tc.nc
    ctx = tc._ctx

    h_ap = nc.dram_tensor("h_nat", [BL, P, SC * H2], BF16, kind="ExternalInput").ap()
    h8_ap = nc.dram_tensor("h8t", [BL, P, KC * S], ZDT, kind="ExternalInput").ap()
    w1_ap = nc.dram_tensor("w1t8", [P, KC * ATT], ZDT, kind="ExternalInput").ap()
    w2_ap = nc.dram_tensor("w2t8", [P, KC * ATT], ZDT, kind="ExternalInput").ap()
    htt_ap = nc.dram_tensor("htt_bf", [P, KC * BL * T], BF16, kind="ExternalInput").ap()
    u_ap = nc.dram_tensor("u_col", [P, TT * 32], BF16, kind="ExternalInput").ap()
    w1b_ap = nc.dram_tensor("w1b_col", [P, TT], F32, kind="ExternalInput").ap()
    mask_ap = nc.dram_tensor("maskrep", [P, NG * S], BF16, kind="ExternalInput").ap()
    out_ap = nc.dram_tensor("out", [BL, H2], F32, kind="ExternalOutput").ap()

    singles = ctx.enter_context(tc.tile_pool(name="singles", bufs=1))
    hT_pool = ctx.enter_context(tc.tile_pool(name="hT", bufs=4))
    a_pool = ctx.enter_context(tc.tile_pool(name="a", bufs=20))
    rows = ctx.enter_context(tc.tile_pool(name="rows", bufs=4))
    z_psum = ctx.enter_context(tc.tile_pool(name="z_ps", bufs=4, space="PSUM"))
    bias_ws_psum = ctx.enter_context(tc.tile_pool(name="bw_ps", bufs=2, space="PSUM"))
    beta_aT_psum = ctx.enter_context(tc.tile_pool(name="ba_ps", bufs=2, space="PSUM"))

    # ================= prologue: warmup, constants, bias =================
    warm = singles.tile([P, S], BF16)
    nc.vector.memset(warm, 0.0)
    warm_ps = bias_ws_psum.tile([P, S], F32, tag="bw")
    for _ in range(WARMUP_MMS):
        nc.tensor.matmul(warm_ps, lhsT=warm[:, 0:P], rhs=warm, start=True, stop=True)

    # scalar HWDGE queue: htt first (bias path), then the per-rep hT stream
    htt_sb = singles.tile([P, KC, BL * T], BF16)
    nc.scalar.dma_start(out=htt_sb, in_=htt_ap.rearrange("p (k j) -> p k j", k=KC))

    # sync HWDGE queue: weights + small constants, ahead of the h_nat stream
    w1t_sb = singles.tile([P, KC, ATT], ZDT)
    nc.sync.dma_start(out=w1t_sb, in_=w1_ap.rearrange("p (k a) -> p k a", k=KC))
    w2t_sb = singles.tile([P, KC, ATT], ZDT)
    nc.sync.dma_start(out=w2t_sb, in_=w2_ap.rearrange("p (k a) -> p k a", k=KC))
    u_sb = singles.tile([P, TT, 32], BF16)
    nc.sync.dma_start(out=u_sb, in_=u_ap.rearrange("p (t r) -> p t r", t=TT))
    w1b_sb = singles.tile([P, TT], F32)
    nc.sync.dma_start(out=w1b_sb, in_=w1b_ap)
    mask_sb = singles.tile([P, NG, S], BF16)
    nc.sync.dma_start(out=mask_sb, in_=mask_ap.rearrange("p (g s) -> p g s", g=NG))
    ident = singles.tile([P, P], BF16)
    make_identity(nc, ident)

    # ht sum -> (fp8) columns; bias_col[t] = (w2*WS @ ht_sum)/(T*WS) + w1_b
    htm = singles.tile([P, KC, BL], BF16)
    for c in range(KC):
        with nc.allow_low_precision("bf16 sum of 32 bf16 values, fp32 internal"):
            nc.vector.reduce_sum(
                out=htm[:, c, :],
                in_=htt_sb[:, c, :].rearrange("p (b t) -> p b t", b=BL),
                axis=mybir.AxisListType.X,
            )
    if FP8_Z:
        htm_z = singles.tile([P, KC, BL], ZDT)
        nc.vector.tensor_copy(out=htm_z, in_=htm)
    else:
        htm_z = htm
    bias_col = singles.tile([P, TT, BL], F32)
    for t in range(TT):
        b2_ps = bias_ws_psum.tile([P, S], F32, tag="bw")
        for c in range(KC):
            nc.tensor.matmul(
                b2_ps[:, 0:BL],
                lhsT=w2t_sb[:, c, t * P : (t + 1) * P],
                rhs=htm_z[:, c, :],
                start=(c == 0),
                stop=(c == KC - 1),
            )
        nc.vector.tensor_scalar(
            out=bias_col[:, t, :],
            in0=b2_ps[:, 0:BL],
            scalar1=1.0 / (T * WS),
            scalar2=w1b_sb[:, t : t + 1],
            op0=mybir.AluOpType.mult,
            op1=mybir.AluOpType.add,
        )

    # ========================== per-rep body ==========================
    # Group 1's tail (beta/softmax/wsum) is deferred across the rep
    # boundary: its PE work is emitted between the NEXT rep's first z
    # matmuls, so the PE never stalls on the softmax chain at the
    # boundary. The final rep's tail is flushed after the loop.
    pending = []

    def emit_rep():
        hT_tiles = [None] * BL
        a_tiles = {}
        beta_tiles = {}

        def load_hT(b):
            hT_b = hT_pool.tile([P, KC, S], ZDT, tag="hT")
            nc.scalar.dma_start(
                out=hT_b, in_=h8_ap[b].rearrange("p (k s) -> p k s", k=KC)
            )
            hT_tiles[b] = hT_b

        load_hT(0)
        load_hT(1)
        load_hT(2)

        # h_nat split per group so the next rep's group-0 stream can begin
        # while this rep's deferred group-1 wsum still reads its own tile
        h_nat_tiles = {}

        def load_hnat(b):
            g, j = divmod(b, 4)
            if g not in h_nat_tiles:
                h_nat_tiles[g] = singles.tile(
                    [P, 4, SC, H2], BF16, tag=f"hnat{g}", name=f"hnat{g}"
                )
            nc.sync.dma_start(
                out=h_nat_tiles[g][:, j],
                in_=h_ap[b].rearrange("p (sc d) -> p sc d", sc=SC),
            )

        load_hnat(0)

        def emit_beta(g):
            # beta for 4 batches, batch 4g+j on partitions 32j..32j+31 (x32)
            beta_ps = beta_aT_psum.tile([P, S], F32, tag="ba")
            # mask term (0 / -1e20, replicated layout) first, via an
            # identity-lhsT matmul, so the last u-matmul completes beta
            nc.tensor.matmul(
                beta_ps,
                lhsT=ident,
                rhs=mask_sb[:, g, :],
                start=True,
                stop=False,
                skip_group_check=True,
            )
            for bb in range(4):
                b = 4 * g + bb
                for t in range(TT):
                    nc.tensor.matmul(
                        beta_ps[32 * bb : 32 * bb + 32, :],
                        lhsT=u_sb[:, t, :],
                        rhs=a_tiles[(b, t)],
                        start=False,
                        stop=(t == TT - 1),
                        tile_position=(0, 32 * bb),
                        skip_group_check=True,
                    )
            beta_tiles[g] = beta_ps

        def emit_tail(g):
            # softmax over S (free dim), all 4 batches (x32 replicas) at once
            beta_ps = beta_tiles[g]
            negmax = rows.tile([P, 1], F32, tag="negmax")
            nc.vector.reduce_max(
                out=negmax, in_=beta_ps, axis=mybir.AxisListType.X, negate=True
            )
            alpha_bf = rows.tile([P, S], BF16, tag="alpha")
            sumrow = rows.tile([P, 1], F32, tag="sumrow")
            nc.scalar.activation(
                out=alpha_bf,
                in_=beta_ps,
                func=mybir.ActivationFunctionType.Exp,
                bias=negmax[:, 0:1],
                scale=1.0,
                accum_out=sumrow[:, 0:1],
            )
            rinv = rows.tile([P, 1], F32, tag="rinv")
            nc.vector.reciprocal(rinv, sumrow)

            # PE transpose: [128(4bx32r), S] -> per sc [128(s), 128(4bx32r)]
            alpha_sb = rows.tile([P, SC, P], BF16, tag="alphasb")
            for sc in range(SC):
                aT_ps = beta_aT_psum.tile([P, P], BF16, tag="ba")
                nc.tensor.transpose(
                    aT_ps, alpha_bf[:, sc * P : (sc + 1) * P], ident
                )
                nc.vector.tensor_copy(out=alpha_sb[:, sc, :], in_=aT_ps)

            # weighted sum (unnormalized), 4 batches in PE column groups;
            # normalization folds into the output copy as per-partition scale
            o_sc = rows.tile([P, NH, 512], F32, tag="orow")
            for nh in range(NH):
                ws_ps = bias_ws_psum.tile([P, 512], F32, tag="bw")
                for bb in range(4):
                    for sc in range(SC):
                        nc.tensor.matmul(
                            ws_ps[32 * bb : 32 * bb + 32, :],
                            lhsT=alpha_sb[:, sc, 32 * bb : 32 * bb + 32],
                            rhs=h_nat_tiles[g][:, bb, sc, nh * 512 : (nh + 1) * 512],
                            start=(sc == 0),
                            stop=(sc == SC - 1),
                            tile_position=(0, 32 * bb),
                        )
                nc.vector.tensor_scalar_mul(o_sc[:, nh, :], ws_ps, rinv[:, 0:1])
            # strided gather: partitions {0,32,64,96} -> out rows, both halves
            nc.gpsimd.dma_start(
                out=out_ap[4 * g : 4 * g + 4, :],
                in_=o_sc.rearrange("(b r) n s -> b r (n s)", r=32)[:, 0, :],
            )

        def emit_z(b, t):
            z_ps = z_psum.tile([P, S], F32, tag="z")
            hT_b = hT_tiles[b]
            if FP8_Z:
                for kk in range(KD):
                    nc.tensor.matmul(
                        z_ps,
                        lhsT=w1t_sb[:, 2 * kk : 2 * kk + 2, t * P : (t + 1) * P],
                        rhs=hT_b[:, 2 * kk : 2 * kk + 2, :],
                        start=(kk == 0),
                        stop=(kk == KD - 1),
                        perf_mode=mybir.MatmulPerfMode.DoubleRow,
                    )
            else:
                for k in range(KC):
                    nc.tensor.matmul(
                        z_ps,
                        lhsT=w1t_sb[:, k, t * P : (t + 1) * P],
                        rhs=hT_b[:, k, :],
                        start=(k == 0),
                        stop=(k == KC - 1),
                    )
            return z_ps

        for b in range(BL):
            if b <= 1 and pending:
                pending.pop(0)()  # previous rep's deferred g1 beta / tail
            if b + 3 < BL:
                load_hT(b + 3)
            if b + 1 < BL:
                load_hnat(b + 1)
            for t in range(TT):
                z_ps = emit_z(b, t)
                a_t = a_pool.tile([P, S], BF16, tag="a")
                nc.scalar.activation(
                    out=a_t,
                    in_=z_ps,
                    func=mybir.ActivationFunctionType.Tanh,
                    bias=bias_col[:, t, b : b + 1],
                    scale=1.0 / WS,
                )
                a_tiles[(b, t)] = a_t
            if b == 4:
                emit_beta(0)
            elif b == 5:
                emit_tail(0)
        pending.append(lambda: emit_beta(1))
        pending.append(lambda: emit_tail(1))

    for _rep in range(reps):
        emit_rep()
    for fn in pending:
        fn()


_CACHE = {}


def build(reps=1):
    key = ("nc", reps)
    if key in _CACHE:
        return _CACHE[key]
    nc = bacc.Bacc("TRN2", target_bir_lowering=False, debug=False)
    with tile.TileContext(nc) as tc:
        with ExitStack() as ctx:
            tc._ctx = ctx
            _body(tc, reps=reps)
    nc.compile()
    _CACHE[key] = nc
    return nc


def _prep_core_inputs(h, h_mask, ht, w1_w, w1_b, u_w):
    """Host-side sharding + layout prep. Returns list of 8 per-core dicts."""
    bf = ml_dtypes.bfloat16
    zdt = NP_F8 if FP8_Z else bf
    h = np.asarray(h, dtype=np.float32)
    ht = np.asarray(ht, dtype=np.float32)

    # h_nat[b, p, sc*H2 + d] = h[b, sc*128+p, d]   (contiguous per partition)
    h_nat = np.ascontiguousarray(
        h.reshape(B, SC, P, H2).transpose(0, 2, 1, 3).reshape(B, P, SC * H2)
    ).astype(bf)
    # h8t[b, p, k*S + s] = h[b, s, k*128+p]
    h8t = np.ascontiguousarray(
        h.transpose(0, 2, 1).reshape(B, KC, P, S).transpose(0, 2, 1, 3)
        .reshape(B, P, KC * S)
    ).astype(zdt)

    def prep_w(w):  # [ATT, H2] -> [P, KC*ATT]: w8[p, k*ATT+a] = w[a, k*128+p]
        wt = np.ascontiguousarray(np.asarray(w, dtype=np.float32).T)  # [H2, ATT]
        return np.ascontiguousarray(
            (wt * WS).reshape(KC, P, ATT).transpose(1, 0, 2).reshape(P, KC * ATT)
        ).astype(zdt)

    w1t8 = prep_w(w1_w[:, :H2])
    w2t8 = prep_w(w1_w[:, H2:])

    u_col = np.ascontiguousarray(
        np.repeat(
            np.asarray(u_w[0], dtype=np.float32).reshape(TT, P).T[:, :, None],
            32,
            axis=2,
        ).reshape(P, TT * 32)
    ).astype(bf)
    w1b_col = np.ascontiguousarray(
        np.asarray(w1_b, dtype=np.float32).reshape(TT, P).T
    ).astype(np.float32)

    neg = np.float32(-1e20)
    maskadd = np.where(np.asarray(h_mask) != 0, np.float32(0.0), neg)  # [B, S]

    in_maps = []
    for core in range(NCORES):
        lo, hi = core * BL, (core + 1) * BL
        htc = ht[lo:hi].reshape(BL * T, H2).T  # [H2, BL*T]
        htt = np.ascontiguousarray(
            htc.reshape(KC, P, BL * T).transpose(1, 0, 2).reshape(P, KC * BL * T)
        ).astype(bf)
        # mrep[32*j+r, g*S+s] = maskadd[lo + 4g+j, s]
        mrep = np.ascontiguousarray(
            np.repeat(maskadd[lo:hi].reshape(NG, 4, 1, S), 32, axis=2)
            .reshape(NG, P, S).transpose(1, 0, 2).reshape(P, NG * S)
        ).astype(bf)
        in_maps.append(
            {
                "h_nat": np.ascontiguousarray(h_nat[lo:hi]),
                "h8t": np.ascontiguousarray(h8t[lo:hi]),
                "w1t8": w1t8,
                "w2t8": w2t8,
                "htt_bf": htt,
                "u_col": u_col,
                "w1b_col": w1b_col,
                "maskrep": mrep,
            }
        )
    return in_maps


def kernel(h, h_mask, ht, w1_w, w1_b, u_w):
    nc = build()
    in_maps = _prep_core_inputs(h, h_mask, ht, w1_w, w1_b, u_w)
    res = bass_utils.run_bass_kernel_spmd(
        nc,
        in_maps,
        core_ids=list(range(NCORES)),
        trace=bool(int(os.environ.get("KERNEL_TRACE", "0"))),
    )
    _CACHE["last_result"] = res
    out = np.concatenate([r["out"] for r in res.results], axis=0)
    return np.ascontiguousarray(out.astype(np.float32))


# revision 23
# speedup vs baseline: 2.6678x; 2.6678x over previous
"""Trainium2 Bass kernel for nn_Attention_3607772529228 (sparse_attention).

Reference computation (B=64, S=512, T=32, 2H=1024, ATT=512):
    ht_mean = mean(ht, axis=1)                               [B, 2H]
    z       = [h ; ht_mean] @ w1_w.T + w1_b                  [B, S, ATT]
    a       = tanh(z)
    beta    = a @ u_w[0];  beta = where(mask, beta, -1e20)   [B, S]
    alpha   = softmax(beta, axis=1)
    out     = einsum('bs,bsd->bd', alpha, h)                 [B, 2H]

Algebraic simplifications (exact):
  * where(valid, ..., 0) on h_cat / a does not affect the output (invalid
    positions only enter through beta, overwritten with -1e20).
  * The ht_mean half of the big matmul folds into a per-batch bias:
    z = h @ w1.T + (w2 @ ht_mean + w1_b).

Distribution: data-parallel over batch B across 8 cores (8 batches/core).

v3 design (the bf16 baseline is HBM-DMA-bound at ~19MB/core/rep):
  * z path in fp8-e4m3: h_t, w1, w2 shipped fp8 (weights pre-scaled x64 to
    dodge the e4m3 subnormal range; folded back via the tanh activation
    scale). z matmuls use DoubleRow (2 fp8 weights/cell, K=256/pass).
    Final rel err ~0.9e-2 (validated vs 2e-2 budget).
  * h_nat (weighted-sum copy) stays bf16 -- output precision needs it.
  * All big DMAs are host-prepped fully contiguous per partition.
  * Constants (w1/w2/htt/u/mask) + the bias matmuls are hoisted out of
    the rep loop and issued before the h_nat stream.
  * beta lands in a 32x-replicated [128, S] layout (4 batches x 32
    replicas); softmax runs in that layout (no gather DMAs). The -1e20
    mask add is one extra identity-lhsT matmul accumulated into beta's
    PSUM. Exp writes bf16 directly (f32 row-sum via accum_out); the
    1/sum normalization is deferred to the output copy (per-partition
    scale), keeping the beta->alpha->transpose chain short.
  * Per-group tail pipelining: group 0's beta/softmax/wsum PE work is
    emitted between group 1's z matmuls; only group 1's tail is exposed.
  * PE warmup matmuls only on rep 0 (cold path).
"""

import os
from contextlib import ExitStack

import numpy as np
import ml_dtypes

import concourse.bass as bass
import concourse.tile as tile
from concourse import bacc, mybir
from concourse import bass_utils
from concourse.masks import make_identity

BF16 = mybir.dt.bfloat16
F8 = mybir.dt.float8e4
F32 = mybir.dt.float32

B, S, T, H2, ATT = 64, 512, 32, 1024, 512
NCORES = 8
BL = B // NCORES  # 8 batches per core
P = 128
KC = H2 // P  # 8 k-chunks over hidden
KD = KC // 2  # 4 DoubleRow k-pairs
TT = ATT // P  # 4 attention tiles
SC = S // P  # 4 sequence chunks
NH = H2 // 512  # 2 output halves
NG = BL // 4  # batch groups of 4 (PE column-group packing)
WARMUP_MMS = 12
FP8_Z = True  # fp8 z path (h_t/w1/w2) with DoubleRow matmuls
WS = 64.0 if FP8_Z else 1.0  # fp8 weight pre-scale (dodges e4m3 subnormals)

ZDT = F8 if FP8_Z else BF16
NP_F8 = ml_dtypes.float8_e4m3


def _body(tc, reps=1):
    nc = tc.nc
    ctx = tc._ctx

    h_ap = nc.dram_tensor("h_nat", [BL, P, SC * H2], BF16, kind="ExternalInput").ap()
    h8_ap = nc.dram_tensor("h8t", [BL, P, KC * S], ZDT, kind="ExternalInput").ap()
    w1_ap = nc.dram_tensor("w1t8", [P, KC * ATT], ZDT, kind="ExternalInput").ap()
    w2_ap = nc.dram_tensor("w2t8", [P, KC * ATT], ZDT, kind="ExternalInput").ap()
    htt_ap = nc.dram_tensor("htt_bf", [P, KC * BL * T], BF16, kind="ExternalInput").ap()
    u_ap = nc.dram_tensor("u_col", [P, TT * 32], BF16, kind="ExternalInput").ap()
    w1b_ap = nc.dram_tensor("w1b_col", [P, TT], F32, kind="ExternalInput").ap()
    mask_ap = nc.dram_tensor("maskrep", [P, NG * S], BF16, kind="ExternalInput").ap()
    out_ap = nc.dram_tensor("out", [BL, H2], F32, kind="ExternalOutput").ap()

    singles = ctx.enter_context(tc.tile_pool(name="singles", bufs=1))
    hT_pool = ctx.enter_context(tc.tile_pool(name="hT", bufs=4))
    a_pool = ctx.enter_context(tc.tile_pool(name="a", bufs=24))
    rows = ctx.enter_context(tc.tile_pool(name="rows", bufs=4))
    z_psum = ctx.enter_context(tc.tile_pool(name="z_ps", bufs=5, space="PSUM"))
    bias_ws_psum = ctx.enter_context(tc.tile_pool(name="bw_ps", bufs=1, space="PSUM"))
    beta_aT_psum = ctx.enter_context(tc.tile_pool(name="ba_ps", bufs=2, space="PSUM"))

    # ================= prologue: warmup, constants, bias =================
    warm = singles.tile([P, S], BF16)
    nc.vector.memset(warm, 0.0)
    warm_ps = bias_ws_psum.tile([P, S], F32, tag="bw")
    for _ in range(WARMUP_MMS):
        nc.tensor.matmul(warm_ps, lhsT=warm[:, 0:P], rhs=warm, start=True, stop=True)

    # scalar HWDGE queue: htt first (bias path), then the per-rep hT stream
    htt_sb = singles.tile([P, KC, BL * T], BF16)
    nc.scalar.dma_start(out=htt_sb, in_=htt_ap.rearrange("p (k j) -> p k j", k=KC))

    # sync HWDGE queue: weights + small constants, ahead of the h_nat stream
    w1t_sb = singles.tile([P, KC, ATT], ZDT)
    nc.sync.dma_start(out=w1t_sb, in_=w1_ap.rearrange("p (k a) -> p k a", k=KC))
    w2t_sb = singles.tile([P, KC, ATT], ZDT)
    nc.sync.dma_start(out=w2t_sb, in_=w2_ap.rearrange("p (k a) -> p k a", k=KC))
    u_sb = singles.tile([P, TT, 32], BF16)
    nc.sync.dma_start(out=u_sb, in_=u_ap.rearrange("p (t r) -> p t r", t=TT))
    w1b_sb = singles.tile([P, TT], F32)
    nc.sync.dma_start(out=w1b_sb, in_=w1b_ap)
    mask_sb = singles.tile([P, NG, S], BF16)
    nc.sync.dma_start(out=mask_sb, in_=mask_ap.rearrange("p (g s) -> p g s", g=NG))
    ident = singles.tile([P, P], BF16)
    make_identity(nc, ident)

    # ht sum -> (fp8) columns; bias_col[t] = (w2*WS @ ht_sum)/(T*WS) + w1_b
    htm = singles.tile([P, KC, BL], BF16)
    for c in range(KC):
        with nc.allow_low_precision("bf16 sum of 32 bf16 values, fp32 internal"):
            nc.vector.reduce_sum(
                out=htm[:, c, :],
                in_=htt_sb[:, c, :].rearrange("p (b t) -> p b t", b=BL),
                axis=mybir.AxisListType.X,
            )
    if FP8_Z:
        htm_z = singles.tile([P, KC, BL], ZDT)
        nc.vector.tensor_copy(out=htm_z, in_=htm)
    else:
        htm_z = htm
    bias_col = singles.tile([P, TT, BL], F32)
    for t in range(TT):
        b2_ps = bias_ws_psum.tile([P, S], F32, tag="bw")
        for c in range(KC):
            nc.tensor.matmul(
                b2_ps[:, 0:BL],
                lhsT=w2t_sb[:, c, t * P : (t + 1) * P],
                rhs=htm_z[:, c, :],
                start=(c == 0),
                stop=(c == KC - 1),
            )
        nc.vector.tensor_scalar(
            out=bias_col[:, t, :],
            in0=b2_ps[:, 0:BL],
            scalar1=1.0 / (T * WS),
            scalar2=w1b_sb[:, t : t + 1],
            op0=mybir.AluOpType.mult,
            op1=mybir.AluOpType.add,
        )

    # ========================== per-rep body ==========================
    # Group 1's tail (beta/softmax/wsum) is deferred across the rep
    # boundary: its PE work is emitted between the NEXT rep's first z
    # matmuls, so the PE never stalls on the softmax chain at the
    # boundary. The final rep's tail is flushed after the loop.
    pending = []

    def emit_rep():
        hT_tiles = [None] * BL
        a_tiles = {}
        beta_tiles = {}

        def load_hT(b):
            hT_b = hT_pool.tile([P, KC, S], ZDT, tag="hT")
            nc.scalar.dma_start(
                out=hT_b, in_=h8_ap[b].rearrange("p (k s) -> p k s", k=KC)
            )
            hT_tiles[b] = hT_b

        load_hT(0)
        load_hT(1)
        load_hT(2)

        # h_nat split per group so the next rep's group-0 stream can begin
        # while this rep's deferred group-1 wsum still reads its own tile
        h_nat_tiles = {}

        def load_hnat(b):
            g, j = divmod(b, 4)
            if g not in h_nat_tiles:
                h_nat_tiles[g] = singles.tile(
                    [P, 4, SC, H2], BF16, tag=f"hnat{g}", name=f"hnat{g}"
                )
            nc.sync.dma_start(
                out=h_nat_tiles[g][:, j],
                in_=h_ap[b].rearrange("p (sc d) -> p sc d", sc=SC),
            )

        load_hnat(0)

        def emit_beta(g):
            # beta for 4 batches, batch 4g+j on partitions 32j..32j+31 (x32)
            beta_ps = beta_aT_psum.tile([P, S], F32, tag="ba")
            # mask term (0 / -1e20, replicated layout) first, via an
            # identity-lhsT matmul, so the last u-matmul completes beta
            nc.tensor.matmul(
                beta_ps,
                lhsT=ident,
                rhs=mask_sb[:, g, :],
                start=True,
                stop=False,
                skip_group_check=True,
            )
            for bb in range(4):
                b = 4 * g + bb
                for t in range(TT):
                    nc.tensor.matmul(
                        beta_ps[32 * bb : 32 * bb + 32, :],
                        lhsT=u_sb[:, t, :],
                        rhs=a_tiles[(b, t)],
                        start=False,
                        stop=(t == TT - 1),
                        tile_position=(0, 32 * bb),
                        skip_group_check=True,
                    )
            beta_tiles[g] = beta_ps

        def emit_tail(g):
            # softmax over S (free dim), all 4 batches (x32 replicas) at once
            beta_ps = beta_tiles[g]
            negmax = rows.tile([P, 1], F32, tag="negmax")
            nc.vector.reduce_max(
                out=negmax, in_=beta_ps, axis=mybir.AxisListType.X, negate=True
            )
            alpha_bf = rows.tile([P, S], BF16, tag="alpha")
            sumrow = rows.tile([P, 1], F32, tag="sumrow")
            nc.scalar.activation(
                out=alpha_bf,
                in_=beta_ps,
                func=mybir.ActivationFunctionType.Exp,
                bias=negmax[:, 0:1],
                scale=1.0,
                accum_out=sumrow[:, 0:1],
            )
            rinv = rows.tile([P, 1], F32, tag="rinv")
            nc.vector.reciprocal(rinv, sumrow)

            # PE transpose: [128(4bx32r), S] -> per sc [128(s), 128(4bx32r)]
            alpha_sb = rows.tile([P, SC, P], BF16, tag="alphasb")
            for sc in range(SC):
                aT_ps = beta_aT_psum.tile([P, P], BF16, tag="ba")
                nc.tensor.transpose(
                    aT_ps, alpha_bf[:, sc * P : (sc + 1) * P], ident
                )
                nc.vector.tensor_copy(out=alpha_sb[:, sc, :], in_=aT_ps)

            # weighted sum (unnormalized), 4 batches in PE column groups;
            # normalization folds into the output copy as per-partition scale
            o_sc = rows.tile([P, NH, 512], F32, tag="orow")
            for nh in range(NH):
                ws_ps = bias_ws_psum.tile([P, 512], F32, tag="bw")
                for bb in range(4):
                    for sc in range(SC):
                        nc.tensor.matmul(
                            ws_ps[32 * bb : 32 * bb + 32, :],
                            lhsT=alpha_sb[:, sc, 32 * bb : 32 * bb + 32],
                            rhs=h_nat_tiles[g][:, bb, sc, nh * 512 : (nh + 1) * 512],
                            start=(sc == 0),
                            stop=(sc == SC - 1),
                            tile_position=(0, 32 * bb),
                        )
                nc.vector.tensor_scalar_mul(o_sc[:, nh, :], ws_ps, rinv[:, 0:1])
            # strided gather: partitions {0,32,64,96} -> out rows, both halves
            nc.gpsimd.dma_start(
                out=out_ap[4 * g : 4 * g + 4, :],
                in_=o_sc.rearrange("(b r) n s -> b r (n s)", r=32)[:, 0, :],
            )

        def emit_z(b, t):
            z_ps = z_psum.tile([P, S], F32, tag="z")
            hT_b = hT_tiles[b]
            if FP8_Z:
                for kk in range(KD):
                    nc.tensor.matmul(
                        z_ps,
                        lhsT=w1t_sb[:, 2 * kk : 2 * kk + 2, t * P : (t + 1) * P],
                        rhs=hT_b[:, 2 * kk : 2 * kk + 2, :],
                        start=(kk == 0),
                        stop=(kk == KD - 1),
                        perf_mode=mybir.MatmulPerfMode.DoubleRow,
                    )
            else:
                for k in range(KC):
                    nc.tensor.matmul(
                        z_ps,
                        lhsT=w1t_sb[:, k, t * P : (t + 1) * P],
                        rhs=hT_b[:, k, :],
                        start=(k == 0),
                        stop=(k == KC - 1),
                    )
            return z_ps

        for b in range(BL):
            if b <= 1 and pending:
                pending.pop(0)()  # previous rep's deferred g1 beta / tail
            if b + 3 < BL:
                load_hT(b + 3)
            if b + 1 < BL:
                load_hnat(b + 1)
            for t in range(TT):
                z_ps = emit_z(b, t)
                a_t = a_pool.tile([P, S], BF16, tag="a")
                nc.scalar.activation(
                    out=a_t,
                    in_=z_ps,
                    func=mybir.ActivationFunctionType.Tanh,
                    bias=bias_col[:, t, b : b + 1],
                    scale=1.0 / WS,
                )
                a_tiles[(b, t)] = a_t
            if b == 4:
                emit_beta(0)
            elif b == 5:
                emit_tail(0)
        pending.append(lambda: emit_beta(1))
        pending.append(lambda: emit_tail(1))

    for _rep in range(reps):
        emit_rep()
    for fn in pending:
        fn()


_CACHE = {}


def build(reps=1):
    key = ("nc", reps)
    if key in _CACHE:
        return _CACHE[key]
    nc = bacc.Bacc("TRN2", target_bir_lowering=False, debug=False)
    with tile.TileContext(nc) as tc:
        with ExitStack() as ctx:
            tc._ctx = ctx
            _body(tc, reps=reps)
    nc.compile()
    _CACHE[key] = nc
    return nc


def _prep_core_inputs(h, h_mask, ht, w1_w, w1_b, u_w):
    """Host-side sharding + layout prep. Returns list of 8 per-core dicts."""
    bf = ml_dtypes.bfloat16
    zdt = NP_F8 if FP8_Z else bf
    h = np.asarray(h, dtype=np.float32)
    ht = np.asarray(ht, dtype=np.float32)

    # h_nat[b, p, sc*H2 + d] = h[b, sc*128+p, d]   (contiguous per partition)
    h_nat = np.ascontiguousarray(
        h.reshape(B, SC, P, H2).transpose(0, 2, 1, 3).reshape(B, P, SC * H2)
    ).astype(bf)
    # h8t[b, p, k*S + s] = h[b, s, k*128+p]
    h8t = np.ascontiguousarray(
        h.transpose(0, 2, 1).reshape(B, KC, P, S).transpose(0, 2, 1, 3)
        .reshape(B, P, KC * S)
    ).astype(zdt)

    def prep_w(w):  # [ATT, H2] -> [P, KC*ATT]: w8[p, k*ATT+a] = w[a, k*128+p]
        wt = np.ascontiguousarray(np.asarray(w, dtype=np.float32).T)  # [H2, ATT]
        return np.ascontiguousarray(
            (wt * WS).reshape(KC, P, ATT).transpose(1, 0, 2).reshape(P, KC * ATT)
        ).astype(zdt)

    w1t8 = prep_w(w1_w[:, :H2])
    w2t8 = prep_w(w1_w[:, H2:])

    u_col = np.ascontiguousarray(
        np.repeat(
            np.asarray(u_w[0], dtype=np.float32).reshape(TT, P).T[:, :, None],
            32,
            axis=2,
        ).reshape(P, TT * 32)
    ).astype(bf)
    w1b_col = np.ascontiguousarray(
        np.asarray(w1_b, dtype=np.float32).reshape(TT, P).T
    ).astype(np.float32)

    neg = np.float32(-1e20)
    maskadd = np.where(np.asarray(h_mask) != 0, np.float32(0.0), neg)  # [B, S]

    in_maps = []
    for core in range(NCORES):
        lo, hi = core * BL, (core + 1) * BL
        htc = ht[lo:hi].reshape(BL * T, H2).T  # [H2, BL*T]
        htt = np.ascontiguousarray(
            htc.reshape(KC, P, BL * T).transpose(1, 0, 2).reshape(P, KC * BL * T)
        ).astype(bf)
        # mrep[32*j+r, g*S+s] = maskadd[lo + 4g+j, s]
        mrep = np.ascontiguousarray(
            np.repeat(maskadd[lo:hi].reshape(NG, 4, 1, S), 32, axis=2)
            .reshape(NG, P, S).transpose(1, 0, 2).reshape(P, NG * S)
        ).astype(bf)
        in_maps.append(
            {
                "h_nat": np.ascontiguousarray(h_nat[lo:hi]),
                "h8t": np.ascontiguousarray(h8t[lo:hi]),
                "w1t8": w1t8,
                "w2t8": w2t8,
                "htt_bf": htt,
                "u_col": u_col,
                "w1b_col": w1b_col,
                "maskrep": mrep,
            }
        )
    return in_maps


def kernel(h, h_mask, ht, w1_w, w1_b, u_w):
    nc = build()
    in_maps = _prep_core_inputs(h, h_mask, ht, w1_w, w1_b, u_w)
    res = bass_utils.run_bass_kernel_spmd(
        nc,
        in_maps,
        core_ids=list(range(NCORES)),
        trace=bool(int(os.environ.get("KERNEL_TRACE", "0"))),
    )
    _CACHE["last_result"] = res
    out = np.concatenate([r["out"] for r in res.results], axis=0)
    return np.ascontiguousarray(out.astype(np.float32))
